# revision 26
# baseline (speedup 1.0000x reference)
"""GatedDeltaNet on 8 trn2 NeuronCores (Bass/Tile).

Sharding: 4 groups x 2 cores. Group g owns heads 3g..3g+2; within the group,
core parity d owns DV-half d (DVS=128 of DV=256) of each head. Per-core work:
  - projections hs @ [Wq|Wk|Wv|Wg|Wb|Wa] slices (feature-major via hsT)
  - causal 4-tap conv + SiLU, l2norm (q,k)
  - chunked gated delta rule, chunk C=128: per (chunk, head) precompute
    (I+A)^-1 by Neumann squaring (nilpotent strict-lower A), then a short
    sequential chain of 2 matmuls per chunk for the state.
  - RMS-norm needs sum-sq over the full DV=256 -> tiny pair-AllReduce.
  - o_proj partial [T, HID] then ReduceScatter(+) over all 8 cores.
Host only slices/casts weights and concatenates the 8 output row-blocks.
"""

import numpy as np
import ml_dtypes
from contextlib import ExitStack

import concourse.bass as bass
import concourse.bacc as bacc
import concourse.mybir as mybir
import concourse.tile as tile
from concourse.bass_utils import run_bass_kernel_spmd

F32 = mybir.dt.float32
BF16 = mybir.dt.bfloat16
AF = mybir.ActivationFunctionType
OP = mybir.AluOpType
BF = ml_dtypes.bfloat16

T, HID = 1024, 2048
H, DK, DV = 12, 128, 256
C, NCH = 128, 8          # chunk size, num chunks
NH, DVS = 3, 128         # heads per core, DV half per core
NCORES = 8
# column layout of the per-core weight matrix W [HID, WCOLS]
QO, KO, VO, GO, BO, AO = 0, 384, 768, 1152, 1536, 1539
WCOLS = 1542
NEG = -1e30
EPS_L2 = 1e-6
NORM_EPS = 1e-5


def build(dbg=False):
    nc = bacc.Bacc("TRN2", debug=False, num_devices=NCORES)
    d_hsT = nc.dram_tensor("hsT", [HID, T], BF16, kind="ExternalInput").ap()
    d_w = nc.dram_tensor("w", [HID, WCOLS], BF16, kind="ExternalInput").ap()
    d_wo = nc.dram_tensor("wo", [NH * DVS, HID], BF16, kind="ExternalInput").ap()
    d_convw = nc.dram_tensor("convw", [1152, 4], F32, kind="ExternalInput").ap()
    d_masks = nc.dram_tensor("masks", [128, 256], F32, kind="ExternalInput").ap()
    d_identb = nc.dram_tensor("identb", [128, 128], BF16, kind="ExternalInput").ap()
    d_identf = nc.dram_tensor("identf", [128, 128], F32, kind="ExternalInput").ap()
    d_onesf = nc.dram_tensor("onesf", [1, 128], F32, kind="ExternalInput").ap()
    d_scal = nc.dram_tensor("scal", [NH, 2], F32, kind="ExternalInput").ap()
    d_out = nc.dram_tensor("out", [T // NCORES, HID], BF16, kind="ExternalOutput").ap()
    if dbg:
        dd = {
            "d_xs": nc.dram_tensor("d_xs", [9 * 128, T], BF16, kind="ExternalOutput").ap(),
            "d_g": nc.dram_tensor("d_g", [3 * 128, T], BF16, kind="ExternalOutput").ap(),
            "d_beta": nc.dram_tensor("d_beta", [NH, T], F32, kind="ExternalOutput").ap(),
            "d_cum": nc.dram_tensor("d_cum", [NH, T], F32, kind="ExternalOutput").ap(),
            "d_qn": nc.dram_tensor("d_qn", [128, NH * 128], BF16, kind="ExternalOutput").ap(),
            "d_kn": nc.dram_tensor("d_kn", [128, NH * 128], BF16, kind="ExternalOutput").ap(),
            "d_vtm": nc.dram_tensor("d_vtm", [128, NH * 128], BF16, kind="ExternalOutput").ap(),
            "d_e": nc.dram_tensor("d_e", [128, 256], F32, kind="ExternalOutput").ap(),
            "d_per": nc.dram_tensor("d_per", [5 * 128, 128], BF16, kind="ExternalOutput").ap(),
            "d_w1": nc.dram_tensor("d_w1", [128, 128], BF16, kind="ExternalOutput").ap(),
            "d_o": nc.dram_tensor("d_o", [24 * 128, DVS], BF16, kind="ExternalOutput").ap(),
            "d_ssqs": nc.dram_tensor("d_ssqs", [128, NCH * NH], F32, kind="ExternalOutput").ap(),
            "d_op": nc.dram_tensor("d_op", [T, HID], BF16, kind="ExternalOutput").ap(),
        }

    with tile.TileContext(nc) as tc, ExitStack() as ctx:
        def pool(name, bufs, space="SBUF"):
            return ctx.enter_context(tc.tile_pool(name=name, bufs=bufs, space=space))

        konst = pool("konst", 1)
        p_hsT = pool("p_hsT", 16)
        p_wk = pool("p_wk", 24)
        p_ps = pool("p_ps", 5, space="PSUM")    # unified [128,<=512] psum
        p_pt = p_ps
        p_pf = pool("p_pf", 3, space="PSUM")    # [128,256] fused-rhs psum
        p_x = pool("p_x", 3)
        p_xs = pool("p_xs", 9)
        p_g = pool("p_g", 1)
        p_small = pool("p_small", 1)
        p_tm = pool("p_tm", 8)
        p_raw = pool("p_raw", 8)
        p_sc = pool("p_sc", 2)
        p_per = pool("p_per", 6)
        p_chain = pool("p_chain", 2)
        p_o = pool("p_o", 24)
        p_out = pool("p_out", 2)
        p_dram = pool("p_dram", 1, space="DRAM")

        # ---- constants ----
        masks = konst.tile([128, 256], F32, tag="masks")
        nc.sync.dma_start(masks, d_masks)
        mask_a, mask_m = masks[:, 0:128], masks[:, 128:256]
        identb = konst.tile([128, 128], BF16, tag="identb")
        nc.sync.dma_start(identb, d_identb)
        identf = konst.tile([128, 128], F32, tag="identf")
        nc.sync.dma_start(identf, d_identf)
        onesf = konst.tile([1, 128], F32, tag="onesf")
        nc.sync.dma_start(onesf, d_onesf)
        scal = konst.tile([NH, 2], F32, tag="scal")
        nc.sync.dma_start(scal, d_scal)
        convw = konst.tile([128, 9, 4], F32, tag="convw")
        nc.sync.dma_start(convw, d_convw.rearrange("(n p) k -> p n k", p=128))
        zeros3 = konst.tile([NH, 128], F32, tag="zeros3")
        nc.gpsimd.memset(zeros3, 0.0)
        epsl2 = konst.tile([128, 1], F32, tag="epsl2")
        nc.gpsimd.memset(epsl2, EPS_L2)
        epsn = konst.tile([128, 1], F32, tag="epsn")
        nc.gpsimd.memset(epsn, NORM_EPS)

        # Wo tiles (needed only at the end, but DMA overlaps)
        wo_sb = [konst.tile([128, HID], BF16, tag=f"wo{j}", name=f"wo{j}")
                 for j in range(NH)]
        for j in range(NH):
            nc.sync.dma_start(wo_sb[j], d_wo[j * 128:(j + 1) * 128, :])

        # ---- load hsT ----
        hsT = []
        for kt in range(16):
            t_ = p_hsT.tile([128, T], BF16, tag="hsT")
            nc.sync.dma_start(t_, d_hsT[kt * 128:(kt + 1) * 128, :])
            hsT.append(t_)

        # ---- projections (feature-major) + conv + silu ----
        # ct 0..2 q, 3..5 k, 6..8 v, 9..11 g, 12 b, 13 a
        X = [None] * 9
        XS = [None] * 9
        G = [None] * 3
        zb = p_small.tile([NH, T], F32, tag="zb")
        za = p_small.tile([NH, T], F32, tag="za")
        for ct in range(14):
            c0 = ct * 128 if ct < 13 else 1539
            cw = 128 if ct < 12 else 3
            wts = []
            for kt in range(16):
                wt = p_wk.tile([128, 128], BF16, tag="wk")
                nc.sync.dma_start(wt[:, 0:cw], d_w[kt * 128:(kt + 1) * 128, c0:c0 + cw])
                wts.append(wt)
            if ct < 9:
                X[ct] = p_x.tile([128, T + 3], BF16, tag="x", name=f"x{ct}")
                nc.gpsimd.memset(X[ct][:, 0:3], 0.0)
            elif ct < 12:
                G[ct - 9] = p_g.tile([128, T], BF16, tag=f"g{ct - 9}", name=f"g{ct - 9}")
            for th in range(2):
                ps = p_ps.tile([cw, 512], F32, tag="pp")
                for kt in range(16):
                    nc.tensor.matmul(ps, wts[kt][:, 0:cw],
                                     hsT[kt][:, th * 512:(th + 1) * 512],
                                     start=(kt == 0), stop=(kt == 15))
                if ct < 9:
                    nc.scalar.copy(X[ct][:, 3 + th * 512: 3 + (th + 1) * 512], ps)
                elif ct < 12:
                    nc.scalar.activation(G[ct - 9][:, th * 512:(th + 1) * 512], ps, AF.Silu)
                elif ct == 12:
                    nc.scalar.copy(zb[:, th * 512:(th + 1) * 512], ps)
                else:
                    nc.scalar.copy(za[:, th * 512:(th + 1) * 512], ps)
            if ct < 9:
                # conv + silu (feature-major, free-dim shifts)
                w4 = convw[:, ct]                      # [128, 4] f32
                ya = p_sc.tile([128, T], BF16, tag="conv_a", name=f"ya{ct}")
                yb = p_sc.tile([128, T], BF16, tag="conv_b", name=f"yb{ct}")
                nc.vector.tensor_scalar_mul(ya, X[ct][:, 0:T], w4[:, 0:1])
                nc.vector.scalar_tensor_tensor(yb, X[ct][:, 1:T + 1], w4[:, 1:2], ya,
                                               op0=OP.mult, op1=OP.add)
                nc.vector.scalar_tensor_tensor(ya, X[ct][:, 2:T + 2], w4[:, 2:3], yb,
                                               op0=OP.mult, op1=OP.add)
                nc.vector.scalar_tensor_tensor(yb, X[ct][:, 3:T + 3], w4[:, 3:4], ya,
                                               op0=OP.mult, op1=OP.add)
                XS[ct] = p_xs.tile([128, T], BF16, tag="xs", name=f"xs{ct}")
                nc.scalar.activation(XS[ct], yb, AF.Silu)

        # ---- beta / g / per-chunk cumsum (feature-major, all base-0) ----
        beta_fm = p_small.tile([NH, T], F32, tag="beta_fm")
        eb = p_small.tile([NH, T], F32, tag="eb")
        nc.scalar.activation(eb, zb, AF.Exp, scale=-1.0)
        nc.vector.tensor_scalar_add(eb, eb, 1.0)
        nc.vector.reciprocal(beta_fm, eb)
        sp = p_small.tile([NH, T], F32, tag="sp")
        nc.scalar.activation(sp, za, AF.Exp, bias=scal[:, 1:2])
        nc.vector.tensor_scalar_add(sp, sp, 1.0)
        nc.scalar.activation(sp, sp, AF.Ln)
        gfm = p_small.tile([NH, T], F32, tag="gfm")
        nc.vector.tensor_scalar_mul(gfm, sp, scal[:, 0:1])
        cum3 = p_small.tile([NH, T], F32, tag="cum3")
        for c in range(NCH):
            sl = slice(c * C, (c + 1) * C)
            nc.vector.tensor_tensor_scan(cum3[:, sl], gfm[:, sl], zeros3,
                                         initial=0.0, op0=OP.add, op1=OP.add)
        # per-(chunk, head) cum rows at partition base 0, laid out (c, j)
        cumcat = p_small.tile([1, NCH * NH * 128], F32, tag="cumcat")
        for c in range(NCH):
            for j in range(NH):
                nc.sync.dma_start(cumcat[0:1, (c * NH + j) * 128:(c * NH + j + 1) * 128],
                                  cum3[j:j + 1, c * C:(c + 1) * C])

        # ---- transposes to token-major per chunk + sumsq accumulation ----
        vtm, gtm, bctm, gam = [], [], [], []
        qraw, kraw = [], []
        ssq48 = p_small.tile([128, NCH * 6], F32, tag="ssq48")
        for c in range(NCH):
            sl = slice(c * C, (c + 1) * C)
            qt = p_raw.tile([128, NH * 128], BF16, tag="qtm")
            kt_ = p_raw.tile([128, NH * 128], BF16, tag="ktm")
            vt = p_tm.tile([128, NH * 128], BF16, tag="vtm")
            gt_ = p_tm.tile([128, NH * 128], BF16, tag="gtm")
            for j, (dst, srcs) in enumerate(((qt, XS[0:3]), (kt_, XS[3:6]),
                                             (vt, XS[6:9]), (gt_, G))):
                for jj in range(NH):
                    ps = p_pt.tile([128, 128], BF16, tag="pp", name="tps")
                    nc.tensor.transpose(ps, srcs[jj][:, sl], identb)
                    nc.vector.tensor_copy(dst[:, jj * 128:(jj + 1) * 128], ps)
            vtm.append(vt); gtm.append(gt_)
            qraw.append(qt); kraw.append(kt_)
            bt = p_tm.tile([128, 6], F32, tag="bctm")
            psb = p_pt.tile([128, 3], F32, tag="pp", name="psb")
            nc.tensor.transpose(psb, beta_fm[:, sl], identf[0:3, 0:3])
            nc.vector.tensor_copy(bt[:, 0:3], psb)
            psc = p_pt.tile([128, 3], F32, tag="pp", name="psc")
            nc.tensor.transpose(psc, cum3[:, sl], identf[0:3, 0:3])
            nc.vector.tensor_copy(bt[:, 3:6], psc)
            bctm.append(bt)
            ga = p_tm.tile([128, NH], F32, tag="gam")
            nc.scalar.activation(ga, bt[:, 3:6], AF.Exp)
            gam.append(ga)
            # sum-of-squares for l2norm: cols c*6 + (q0,q1,q2,k0,k1,k2)
            for i, (raw, jj) in enumerate([(qt, 0), (qt, 1), (qt, 2),
                                           (kt_, 0), (kt_, 1), (kt_, 2)]):
                scr = p_sc.tile([128, 128], BF16, tag="sq_scr")
                nc.scalar.activation(scr, raw[:, jj * 128:(jj + 1) * 128], AF.Square,
                                     accum_out=ssq48[:, c * 6 + i:c * 6 + i + 1])

        # batched l2norm factors: rfac = (ssq + eps) ** -0.5, all chunks at once
        lfac = p_small.tile([128, NCH * 6], F32, tag="lfac")
        nc.scalar.activation(lfac, ssq48, AF.Ln, bias=epsl2)
        rfac = p_small.tile([128, NCH * 6], F32, tag="rfac")
        nc.scalar.activation(rfac, lfac, AF.Exp, scale=-0.5)

        qn, kn = qraw, kraw
        for c in range(NCH):
            for j in range(NH):
                sl2 = slice(j * 128, (j + 1) * 128)
                nc.vector.tensor_scalar_mul(qraw[c][:, sl2], qraw[c][:, sl2],
                                            rfac[:, c * 6 + j:c * 6 + j + 1])
                nc.vector.tensor_scalar_mul(kraw[c][:, sl2], kraw[c][:, sl2],
                                            rfac[:, c * 6 + 3 + j:c * 6 + 4 + j])
            if dbg and c == 0:
                nc.sync.dma_start(dd["d_qn"], qraw[0])
                nc.sync.dma_start(dd["d_kn"], kraw[0])
                nc.sync.dma_start(dd["d_vtm"], vtm[0])

        if dbg:
            for i in range(9):
                nc.sync.dma_start(dd["d_xs"][i * 128:(i + 1) * 128, :], XS[i])
            for i in range(3):
                nc.sync.dma_start(dd["d_g"][i * 128:(i + 1) * 128, :], G[i])
            nc.sync.dma_start(dd["d_beta"], beta_fm)
            nc.sync.dma_start(dd["d_cum"], cum3)

        # ---- per-(chunk, head) precompute + chains ----
        S = []
        for j in range(NH):
            s_t = p_chain.tile([128, DVS], BF16, tag=f"S{j}")
            nc.gpsimd.memset(s_t, 0.0)
            S.append(s_t)
        ssq_sb = []

        ssq_in = p_dram.tile([T, NH], F32, tag="ssq_in")
        ssq_out = p_dram.tile([T, NH], F32, tag="ssq_out")

        for c in range(NCH):
            ssq_c = p_tm.tile([128, NH], F32, tag="ssq")
            ssq_sb.append(ssq_c)
            # E broadcast for all 3 heads of this chunk in one K=1 matmul
            e_ps = p_ps.tile([128, NH * 128], F32, tag="pp", name="e_ps")
            nc.tensor.matmul(e_ps, onesf,
                             cumcat[0:1, c * NH * 128:(c + 1) * NH * 128],
                             start=True, stop=True)
            per_j = []
            for j in range(NH):
                sl2 = slice(j * 128, (j + 1) * 128)
                gamma_col = gam[c][:, j:j + 1]
                beta_col = bctm[c][:, j:j + 1]
                cum_col = bctm[c][:, 3 + j:4 + j]

                gq = p_sc.tile([128, 128], BF16, tag="gq")
                nc.vector.tensor_scalar_mul(gq, qn[c][:, sl2], gamma_col)
                # gkv = [gamma*K | V] for the fused TK|W1 matmul
                gkv = p_sc.tile([128, 256], BF16, tag="gkv")
                nc.vector.tensor_scalar_mul(gkv[:, 0:128], kn[c][:, sl2], gamma_col)
                nc.vector.tensor_copy(gkv[:, 128:256], vtm[c][:, sl2])
                kb = p_sc.tile([128, 128], BF16, tag="kb")
                nc.vector.tensor_scalar_mul(kb, kn[c][:, sl2], beta_col)

                def tr(src, tag, ident=identb):
                    ps = p_pt.tile([128, 128], BF16, tag="pp", name="trps")
                    nc.tensor.transpose(ps, src, ident)
                    out = p_sc.tile([128, 128], BF16, tag=tag)
                    nc.vector.tensor_copy(out, ps)
                    return out

                q_fm = tr(qn[c][:, sl2], "q_fm")
                k_fm = tr(kn[c][:, sl2], "k_fm")
                kb_fm = tr(kb, "kb_fm")
                gq_fm = tr(gq, "gq_fm")

                kk_ps = p_pt.tile([128, 128], F32, tag="pp")
                nc.tensor.matmul(kk_ps, k_fm, kb_fm, start=True, stop=True)
                kq_ps = p_pt.tile([128, 128], F32, tag="pp")
                nc.tensor.matmul(kq_ps, kb_fm, q_fm, start=True, stop=True)

                tmp = p_sc.tile([128, 128], F32, tag="tmp")
                nc.vector.tensor_scalar_sub(tmp, e_ps[:, sl2], cum_col)  # cum_j - cum_i
                xa = p_sc.tile([128, 128], F32, tag="xa")
                nc.vector.scalar_tensor_tensor(xa, tmp, -1.0, mask_a,
                                               op0=OP.mult, op1=OP.add)
                ea = p_sc.tile([128, 128], F32, tag="ea")
                nc.scalar.activation(ea, xa, AF.Exp)
                xm = p_sc.tile([128, 128], F32, tag="xm")
                nc.vector.scalar_tensor_tensor(xm, tmp, 1.0, mask_m,
                                               op0=OP.mult, op1=OP.add)
                em = p_sc.tile([128, 128], F32, tag="em")
                nc.scalar.activation(em, xm, AF.Exp)

                B_ = p_sc.tile([128, 128], BF16, tag="B")
                nc.vector.scalar_tensor_tensor(B_, ea, -1.0, kk_ps,
                                               op0=OP.mult, op1=OP.mult)
                # mtku = [Mt | Ku] for the fused (TK^T Mt | TK^T Ku) matmul
                mtku = p_per.tile([128, 256], BF16, tag="mtku")
                nc.vector.tensor_mul(mtku[:, 0:128], em, kq_ps)
                s_col = p_sc.tile([128, 1], F32, tag="s_col")
                nc.vector.tensor_mul(s_col, beta_col, em[:, 127:128])
                nc.vector.tensor_scalar_mul(mtku[:, 128:256], kn[c][:, sl2], s_col)
                gc_col = p_sc.tile([128, 1], F32, tag="gc_col")
                nc.vector.tensor_mul(gc_col, em[:, 127:128], gamma_col)

                # Neumann squaring, 7 factors (I+B^(2^L)), L=0..6.
                # pair tile pr = [Bp^T | Rt]; Rt starts at I.
                # m2 = matmul(lhsT=Bp, rhs=pr) = [(Bp^2)^T | Bp^T Rt] fuses the
                # transpose-squaring with the Rt update.
                bt_ps = p_pt.tile([128, 128], BF16, tag="pp", name="btps")
                nc.tensor.transpose(bt_ps, B_, identb)
                pr = p_sc.tile([128, 256], BF16, tag="pr0")
                nc.vector.tensor_copy(pr[:, 0:128], bt_ps)
                nc.vector.tensor_copy(pr[:, 128:256], identb)
                bp = B_
                for lvl in range(7):
                    bpt, rt = pr[:, 0:128], pr[:, 128:256]
                    if lvl < 6:
                        b2 = p_pt.tile([128, 128], F32, tag="pp")
                        nc.tensor.matmul(b2, bpt, bp, start=True, stop=True)
                        prp = p_pf.tile([128, 256], F32, tag="pf", name="prp")
                        nc.tensor.matmul(prp, bp, pr, start=True, stop=True)
                        pr2 = p_sc.tile([128, 256], BF16, tag=f"pr{lvl + 1}")
                        nc.vector.tensor_copy(pr2[:, 0:128], prp[:, 0:128])
                        nc.vector.tensor_add(pr2[:, 128:256], rt, prp[:, 128:256])
                        bp2 = p_sc.tile([128, 128], BF16, tag=f"bp{lvl}")
                        if lvl % 2 == 0:
                            nc.scalar.copy(bp2, b2)
                        else:
                            nc.vector.tensor_copy(bp2, b2)
                        pr, bp = pr2, bp2
                    else:
                        pps = p_pt.tile([128, 128], F32, tag="pp")
                        nc.tensor.matmul(pps, bp, rt, start=True, stop=True)
                        rt2 = p_sc.tile([128, 128], BF16, tag="rt7")
                        nc.vector.tensor_add(rt2, rt, pps)
                tinvT = rt2

                # fused TK|W1 = Tinv @ [gamma K | V]
                tkw_ps = p_pf.tile([128, 256], F32, tag="pf", name="tkwp")
                nc.tensor.matmul(tkw_ps, tinvT, gkv, start=True, stop=True)
                tkw = p_per.tile([128, 256], BF16, tag="tkw")
                nc.vector.tensor_copy(tkw, tkw_ps)
                tk, w1 = tkw[:, 0:128], tkw[:, 128:256]
                # fused TK^T @ [Mt | Ku]
                mg_ps = p_pf.tile([128, 256], F32, tag="pf", name="mgp")
                nc.tensor.matmul(mg_ps, tk, mtku, start=True, stop=True)
                qeffT = p_per.tile([128, 128], BF16, tag="qeffT")
                nc.vector.tensor_sub(qeffT, gq_fm, mg_ps[:, 0:128])
                gt2 = p_per.tile([128, 128], BF16, tag="gt2")
                nc.vector.scalar_tensor_tensor(gt2, identb, gc_col, mg_ps[:, 128:256],
                                               op0=OP.mult, op1=OP.subtract)
                per_j.append((qeffT, gt2, mtku, w1))
                if dbg and c == 0 and j == 0:
                    nc.sync.dma_start(dd["d_e"][:, 0:128], ea)
                    nc.sync.dma_start(dd["d_e"][:, 128:256], em)
                    for di, dt_ in enumerate((tinvT, qeffT, gt2,
                                              mtku[:, 128:256], mtku[:, 0:128])):
                        nc.sync.dma_start(dd["d_per"][di * 128:(di + 1) * 128, :], dt_)
                    nc.sync.dma_start(dd["d_w1"], w1)

            # chains for chunk c
            for j in range(NH):
                qeffT, gt2, mtku, w1 = per_j[j]
                o_ps = p_pt.tile([128, DVS], F32, tag="pp")
                nc.tensor.matmul(o_ps, qeffT, S[j], start=True, stop=False)
                nc.tensor.matmul(o_ps, mtku[:, 0:128], w1, start=False, stop=True)
                s_ps = p_pt.tile([128, DVS], F32, tag="pp")
                nc.tensor.matmul(s_ps, gt2, S[j], start=True, stop=False)
                nc.tensor.matmul(s_ps, mtku[:, 128:256], w1, start=False, stop=True)
                s_new = p_chain.tile([128, DVS], BF16, tag=f"S{j}")
                nc.vector.tensor_copy(s_new, s_ps)
                S[j] = s_new
                o_sb = p_o.tile([128, DVS], BF16, tag="o_sb")
                nc.scalar.copy(o_sb, o_ps)
                if dbg:
                    nc.sync.dma_start(
                        dd["d_o"][(c * NH + j) * 128:(c * NH + j + 1) * 128, :], o_sb)
                per_j[j] = o_sb
                scr = p_sc.tile([128, DVS], BF16, tag="sq_scr2")
                nc.scalar.activation(scr, o_ps, AF.Square,
                                     accum_out=ssq_c[:, j:j + 1])
            ssq_sb[c] = (ssq_c, per_j)
            nc.sync.dma_start(ssq_in[c * 128:(c + 1) * 128, :], ssq_c)
            # per-chunk pair AllReduce of this chunk's sum-squares
            nc.gpsimd.collective_compute(
                "AllReduce", OP.add,
                replica_groups=[[0, 1], [2, 3], [4, 5], [6, 7]],
                ins=[ssq_in[c * 128:(c + 1) * 128, :].opt()],
                outs=[ssq_out[c * 128:(c + 1) * 128, :].opt()],
            )

        if dbg:
            ssqs = p_small.tile([128, NCH, NH], F32, tag="ssqs")
            nc.sync.dma_start(ssqs, ssq_out.rearrange("(c p) h -> p c h", p=128))
            nc.sync.dma_start(dd["d_ssqs"], ssqs.rearrange("p c h -> p (c h)"))

        # ---- norm * gate, transpose, out proj, reduce-scatter (per chunk) ----
        op_in = p_dram.tile([T, HID], BF16, tag="op_in")
        rs_out = p_dram.tile([T // NCORES, HID], BF16, tag="rs_out")
        for c in range(NCH):
            _, o_list = ssq_sb[c]
            sqc = p_sc.tile([128, NH], F32, tag="sqc")
            nc.sync.dma_start(sqc, ssq_out[c * 128:(c + 1) * 128, :])
            nln = p_sc.tile([128, NH], F32, tag="nln")
            nc.scalar.activation(nln, sqc, AF.Ln, scale=1.0 / DV, bias=epsn)
            nrf = p_sc.tile([128, NH], F32, tag="nrf")
            nc.scalar.activation(nrf, nln, AF.Exp, scale=-0.5)
            ogT = []
            for j in range(NH):
                o_n = p_sc.tile([128, DVS], BF16, tag="o_n")
                nc.vector.tensor_scalar_mul(o_n, o_list[j],
                                            nrf[:, j:j + 1])
                o_g = p_sc.tile([128, DVS], BF16, tag="o_g")
                nc.vector.tensor_mul(o_g, o_n, gtm[c][:, j * 128:(j + 1) * 128])
                ps = p_pt.tile([128, 128], BF16, tag="pp", name="ogps")
                nc.tensor.transpose(ps, o_g, identb)
                og = p_sc.tile([128, 128], BF16, tag=f"ogT{j}")
                nc.vector.tensor_copy(og, ps)
                ogT.append(og)
            outp = p_out.tile([128, HID], BF16, tag="outp")
            for nt in range(4):
                ps = p_ps.tile([128, 512], F32, tag="pp")
                for j in range(NH):
                    nc.tensor.matmul(ps, ogT[j], wo_sb[j][:, nt * 512:(nt + 1) * 512],
                                     start=(j == 0), stop=(j == NH - 1))
                nc.scalar.copy(outp[:, nt * 512:(nt + 1) * 512], ps)
            nc.sync.dma_start(op_in[c * 128:(c + 1) * 128, :], outp)
            if dbg:
                nc.sync.dma_start(dd["d_op"][c * 128:(c + 1) * 128, :], outp)

        nc.gpsimd.collective_compute(
            "ReduceScatter", OP.add,
            replica_groups=[list(range(NCORES))],
            ins=[op_in.opt()], outs=[rs_out.opt()],
        )
        nc.sync.dma_start(d_out, rs_out[:])

    nc.compile()
    return nc


_BUILT = None


def _get_built():
    global _BUILT
    if _BUILT is None:
        _BUILT = build()
    return _BUILT


def _prep_in_maps(hidden_states, Wq, Wk, Wv, Wb, Wa, Wg, Wo,
                  conv_wq, conv_wk, conv_wv, A_log, dt_bias, norm_w):
    hsT = np.ascontiguousarray(hidden_states[0].astype(np.float32).T).astype(BF)
    Wq_b, Wk_b = Wq.astype(BF), Wk.astype(BF)
    Wv_b, Wg_b = Wv.astype(BF), Wg.astype(BF)
    Wb_b, Wa_b = Wb.astype(BF), Wa.astype(BF)
    Wo_eff = (np.tile(norm_w.astype(np.float32), H)[:, None] * Wo).astype(BF)
    cq = conv_wq.astype(np.float32)
    ck = conv_wk.astype(np.float32)
    cv = conv_wv.astype(np.float32)

    ii, jj = np.indices((128, 128))
    mask_a = np.where(jj < ii, 0.0, NEG).astype(np.float32)   # strict lower keep
    mask_m = np.where(jj >= ii, 0.0, NEG).astype(np.float32)  # upper incl diag keep
    masks = np.ascontiguousarray(np.concatenate([mask_a, mask_m], axis=1))
    identb = np.eye(128, dtype=np.float32).astype(BF)
    identf = np.eye(128, dtype=np.float32)
    onesf = np.ones((1, 128), np.float32)

    in_maps = []
    for core in range(NCORES):
        g_, d = divmod(core, 2)
        heads = [3 * g_, 3 * g_ + 1, 3 * g_ + 2]
        qk_idx = np.concatenate([np.arange(h * DK, (h + 1) * DK) for h in heads])
        v_idx = np.concatenate(
            [np.arange(h * DV + d * DVS, h * DV + d * DVS + DVS) for h in heads])
        w = np.concatenate([Wq_b[:, qk_idx], Wk_b[:, qk_idx],
                            Wv_b[:, v_idx], Wg_b[:, v_idx],
                            Wb_b[:, heads], Wa_b[:, heads]], axis=1)
        convw = np.ascontiguousarray(
            np.concatenate([cq[qk_idx], ck[qk_idx], cv[v_idx]], axis=0))
        scal = np.stack([-np.exp(A_log[heads].astype(np.float32)),
                         dt_bias[heads].astype(np.float32)], axis=1)
        in_maps.append({
            "hsT": hsT, "w": np.ascontiguousarray(w),
            "wo": np.ascontiguousarray(Wo_eff[v_idx]),
            "convw": convw, "masks": masks, "identb": identb,
            "identf": identf, "onesf": onesf,
            "scal": np.ascontiguousarray(scal),
        })
    return in_maps


def _run(in_maps, trace=False):
    nc = _get_built()
    return run_bass_kernel_spmd(nc, in_maps, list(range(NCORES)), trace=trace)


def kernel(**inputs):
    in_maps = _prep_in_maps(**inputs)
    res = _run(in_maps, trace=False)
    out = np.concatenate([np.asarray(res.results[i]["out"]) for i in range(NCORES)],
                         axis=0).astype(np.float32)
    return out.reshape(1, T, HID)


_BUILT_DBG = None


def kernel_debug(**inputs):
    global _BUILT_DBG
    if _BUILT_DBG is None:
        _BUILT_DBG = build(dbg=True)
    in_maps = _prep_in_maps(**inputs)
    res = run_bass_kernel_spmd(_BUILT_DBG, in_maps, list(range(NCORES)))
    return res.results


def _ensure_ntff_hook():
    import sys as _sys
    import types as _types
    try:
        from antenv.axon_hooks import get_axon_ntff_profile_hook  # noqa: F401
        return
    except ImportError:
        pass
    from trn_agent_boot.trn_boot import _ntff_profile_via_ctypes
    hook = _ntff_profile_via_ctypes('/opt/axon/libaxon_pjrt.so')
    mod = _types.ModuleType("antenv.axon_hooks")
    mod.get_axon_ntff_profile_hook = lambda: hook
    mod.set_axon_ntff_profile_hook = lambda h: None
    _sys.modules["antenv.axon_hooks"] = mod
    import antenv
    antenv.axon_hooks = mod
    # avoid artifact upload (no bucket access in this env)
    import concourse.bass_utils as _bu
    _bu.upload_artifacts = lambda d: ""


def kernel_traced(**inputs):
    _ensure_ntff_hook()
    in_maps = _prep_in_maps(**inputs)
    res = _run(in_maps, trace=True)
    out = np.concatenate([np.asarray(res.results[i]["out"]) for i in range(NCORES)],
                         axis=0).astype(np.float32)
    return out.reshape(1, T, HID), res


# revision 27
# speedup vs baseline: 1.0624x; 1.0624x over previous
"""GatedDeltaNet on 8 trn2 NeuronCores (Bass/Tile).

Sharding: 4 groups x 2 cores. Group g owns heads 3g..3g+2; within the group,
core parity d owns DV-half d (DVS=128 of DV=256) of each head. Per-core work:
  - projections hs @ [Wq|Wk|Wv|Wg|Wb|Wa] slices (feature-major via hsT)
  - causal 4-tap conv + SiLU, l2norm (q,k)
  - chunked gated delta rule, chunk C=128: per (chunk, head) precompute
    (I+A)^-1 by Neumann squaring (nilpotent strict-lower A), then a short
    sequential chain of 2 matmuls per chunk for the state.
  - RMS-norm needs sum-sq over the full DV=256 -> tiny pair-AllReduce.
  - o_proj partial [T, HID] then ReduceScatter(+) over all 8 cores.
Host only slices/casts weights and concatenates the 8 output row-blocks.
"""

import numpy as np
import ml_dtypes
from contextlib import ExitStack

import concourse.bass as bass
import concourse.bacc as bacc
import concourse.mybir as mybir
import concourse.tile as tile
from concourse.bass_utils import run_bass_kernel_spmd

F32 = mybir.dt.float32
BF16 = mybir.dt.bfloat16
AF = mybir.ActivationFunctionType
OP = mybir.AluOpType
BF = ml_dtypes.bfloat16

T, HID = 1024, 2048
H, DK, DV = 12, 128, 256
C, NCH = 128, 8          # chunk size, num chunks
NH, DVS = 3, 128         # heads per core, DV half per core
NCORES = 8
# column layout of the per-core weight matrix W [HID, WCOLS]
QO, KO, VO, GO, BO, AO = 0, 384, 768, 1152, 1536, 1539
WCOLS = 1542
NEG = -1e30
EPS_L2 = 1e-6
NORM_EPS = 1e-5


def build(dbg=False):
    nc = bacc.Bacc("TRN2", debug=False, num_devices=NCORES)
    d_hsT = nc.dram_tensor("hsT", [HID, T], BF16, kind="ExternalInput").ap()
    d_w = nc.dram_tensor("w", [HID, WCOLS], BF16, kind="ExternalInput").ap()
    d_wo = nc.dram_tensor("wo", [NH * DVS, HID], BF16, kind="ExternalInput").ap()
    d_convw = nc.dram_tensor("convw", [1152, 4], F32, kind="ExternalInput").ap()
    d_masks = nc.dram_tensor("masks", [128, 256], F32, kind="ExternalInput").ap()
    d_identb = nc.dram_tensor("identb", [128, 128], BF16, kind="ExternalInput").ap()
    d_identf = nc.dram_tensor("identf", [128, 128], F32, kind="ExternalInput").ap()
    d_onesf = nc.dram_tensor("onesf", [1, 128], F32, kind="ExternalInput").ap()
    d_scal = nc.dram_tensor("scal", [NH, 2], F32, kind="ExternalInput").ap()
    d_out = nc.dram_tensor("out", [T // NCORES, HID], BF16, kind="ExternalOutput").ap()
    if dbg:
        dd = {
            "d_xs": nc.dram_tensor("d_xs", [9 * 128, T], BF16, kind="ExternalOutput").ap(),
            "d_g": nc.dram_tensor("d_g", [3 * 128, T], BF16, kind="ExternalOutput").ap(),
            "d_beta": nc.dram_tensor("d_beta", [NH, T], F32, kind="ExternalOutput").ap(),
            "d_cum": nc.dram_tensor("d_cum", [NH, T], F32, kind="ExternalOutput").ap(),
            "d_qn": nc.dram_tensor("d_qn", [128, NH * 128], BF16, kind="ExternalOutput").ap(),
            "d_kn": nc.dram_tensor("d_kn", [128, NH * 128], BF16, kind="ExternalOutput").ap(),
            "d_vtm": nc.dram_tensor("d_vtm", [128, NH * 128], BF16, kind="ExternalOutput").ap(),
            "d_e": nc.dram_tensor("d_e", [128, 256], F32, kind="ExternalOutput").ap(),
            "d_per": nc.dram_tensor("d_per", [5 * 128, 128], BF16, kind="ExternalOutput").ap(),
            "d_w1": nc.dram_tensor("d_w1", [128, 128], BF16, kind="ExternalOutput").ap(),
            "d_o": nc.dram_tensor("d_o", [24 * 128, DVS], BF16, kind="ExternalOutput").ap(),
            "d_ssqs": nc.dram_tensor("d_ssqs", [128, NCH * NH], F32, kind="ExternalOutput").ap(),
            "d_op": nc.dram_tensor("d_op", [T, HID], BF16, kind="ExternalOutput").ap(),
        }

    with tile.TileContext(nc) as tc, ExitStack() as ctx:
        def pool(name, bufs, space="SBUF"):
            return ctx.enter_context(tc.tile_pool(name=name, bufs=bufs, space=space))

        konst = pool("konst", 1)
        p_hsT = pool("p_hsT", 16)
        p_wk = pool("p_wk", 24)
        p_ps = pool("p_ps", 5, space="PSUM")    # unified [128,<=512] psum
        p_pt = p_ps
        p_pf = pool("p_pf", 3, space="PSUM")    # [128,256] fused-rhs psum
        p_x = pool("p_x", 3)
        p_xs = pool("p_xs", 9)
        p_g = pool("p_g", 1)
        p_small = pool("p_small", 1)
        p_tm = pool("p_tm", 8)
        p_raw = pool("p_raw", 8)
        p_sc = pool("p_sc", 2)
        p_per = pool("p_per", 6)
        p_chain = pool("p_chain", 2)
        p_o = pool("p_o", 24)
        p_out = pool("p_out", 2)
        p_dram = pool("p_dram", 1, space="DRAM")

        # ---- constants ----
        masks = konst.tile([128, 256], F32, tag="masks")
        nc.sync.dma_start(masks, d_masks)
        mask_a, mask_m = masks[:, 0:128], masks[:, 128:256]
        identb = konst.tile([128, 128], BF16, tag="identb")
        nc.sync.dma_start(identb, d_identb)
        identf = konst.tile([128, 128], F32, tag="identf")
        nc.sync.dma_start(identf, d_identf)
        onesf = konst.tile([1, 128], F32, tag="onesf")
        nc.sync.dma_start(onesf, d_onesf)
        scal = konst.tile([NH, 2], F32, tag="scal")
        nc.sync.dma_start(scal, d_scal)
        convw = konst.tile([128, 9, 4], F32, tag="convw")
        nc.sync.dma_start(convw, d_convw.rearrange("(n p) k -> p n k", p=128))
        zeros3 = konst.tile([NH, 128], F32, tag="zeros3")
        nc.gpsimd.memset(zeros3, 0.0)
        epsl2 = konst.tile([128, 1], F32, tag="epsl2")
        nc.gpsimd.memset(epsl2, EPS_L2)
        epsn = konst.tile([128, 1], F32, tag="epsn")
        nc.gpsimd.memset(epsn, NORM_EPS)

        # Wo tiles (needed only at the end, but DMA overlaps)
        wo_sb = [konst.tile([128, HID], BF16, tag=f"wo{j}", name=f"wo{j}")
                 for j in range(NH)]
        for j in range(NH):
            nc.sync.dma_start(wo_sb[j], d_wo[j * 128:(j + 1) * 128, :])

        # ---- load hsT ----
        hsT = []
        for kt in range(16):
            t_ = p_hsT.tile([128, T], BF16, tag="hsT")
            nc.sync.dma_start(t_, d_hsT[kt * 128:(kt + 1) * 128, :])
            hsT.append(t_)

        # ---- projections (feature-major) + conv + silu ----
        # ct 0..2 q, 3..5 k, 6..8 v, 9..11 g, 12 b, 13 a
        X = [None] * 9
        XS = [None] * 9
        G = [None] * 3
        zb = p_small.tile([NH, T], F32, tag="zb")
        za = p_small.tile([NH, T], F32, tag="za")
        for ct in range(14):
            c0 = ct * 128 if ct < 13 else 1539
            cw = 128 if ct < 12 else 3
            wts = []
            for kt in range(16):
                wt = p_wk.tile([128, 128], BF16, tag="wk")
                nc.sync.dma_start(wt[:, 0:cw], d_w[kt * 128:(kt + 1) * 128, c0:c0 + cw])
                wts.append(wt)
            if ct < 9:
                X[ct] = p_x.tile([128, T + 3], BF16, tag="x", name=f"x{ct}")
                nc.gpsimd.memset(X[ct][:, 0:3], 0.0)
            elif ct < 12:
                G[ct - 9] = p_g.tile([128, T], BF16, tag=f"g{ct - 9}", name=f"g{ct - 9}")
            for th in range(2):
                ps = p_ps.tile([cw, 512], F32, tag="pp")
                for kt in range(16):
                    nc.tensor.matmul(ps, wts[kt][:, 0:cw],
                                     hsT[kt][:, th * 512:(th + 1) * 512],
                                     start=(kt == 0), stop=(kt == 15))
                if ct < 9:
                    nc.scalar.copy(X[ct][:, 3 + th * 512: 3 + (th + 1) * 512], ps)
                elif ct < 12:
                    nc.scalar.activation(G[ct - 9][:, th * 512:(th + 1) * 512], ps, AF.Silu)
                elif ct == 12:
                    nc.scalar.copy(zb[:, th * 512:(th + 1) * 512], ps)
                else:
                    nc.scalar.copy(za[:, th * 512:(th + 1) * 512], ps)
            if ct < 9:
                # conv + silu (feature-major, free-dim shifts)
                w4 = convw[:, ct]                      # [128, 4] f32
                ya = p_sc.tile([128, T], BF16, tag="conv_a", name=f"ya{ct}")
                yb = p_sc.tile([128, T], BF16, tag="conv_b", name=f"yb{ct}")
                nc.vector.tensor_scalar_mul(ya, X[ct][:, 0:T], w4[:, 0:1])
                nc.vector.scalar_tensor_tensor(yb, X[ct][:, 1:T + 1], w4[:, 1:2], ya,
                                               op0=OP.mult, op1=OP.add)
                nc.vector.scalar_tensor_tensor(ya, X[ct][:, 2:T + 2], w4[:, 2:3], yb,
                                               op0=OP.mult, op1=OP.add)
                nc.vector.scalar_tensor_tensor(yb, X[ct][:, 3:T + 3], w4[:, 3:4], ya,
                                               op0=OP.mult, op1=OP.add)
                XS[ct] = p_xs.tile([128, T], BF16, tag="xs", name=f"xs{ct}")
                nc.scalar.activation(XS[ct], yb, AF.Silu)

        # ---- beta / g / per-chunk cumsum (feature-major, all base-0) ----
        beta_fm = p_small.tile([NH, T], F32, tag="beta_fm")
        eb = p_small.tile([NH, T], F32, tag="eb")
        nc.scalar.activation(eb, zb, AF.Exp, scale=-1.0)
        nc.vector.tensor_scalar_add(eb, eb, 1.0)
        nc.vector.reciprocal(beta_fm, eb)
        sp = p_small.tile([NH, T], F32, tag="sp")
        nc.scalar.activation(sp, za, AF.Exp, bias=scal[:, 1:2])
        nc.vector.tensor_scalar_add(sp, sp, 1.0)
        nc.scalar.activation(sp, sp, AF.Ln)
        gfm = p_small.tile([NH, T], F32, tag="gfm")
        nc.vector.tensor_scalar_mul(gfm, sp, scal[:, 0:1])
        cum3 = p_small.tile([NH, T], F32, tag="cum3")
        for c in range(NCH):
            sl = slice(c * C, (c + 1) * C)
            nc.vector.tensor_tensor_scan(cum3[:, sl], gfm[:, sl], zeros3,
                                         initial=0.0, op0=OP.add, op1=OP.add)
        # per-(chunk, head) cum rows at partition base 0, laid out (c, j)
        cumcat = p_small.tile([1, NCH * NH * 128], F32, tag="cumcat")
        for c in range(NCH):
            for j in range(NH):
                nc.sync.dma_start(cumcat[0:1, (c * NH + j) * 128:(c * NH + j + 1) * 128],
                                  cum3[j:j + 1, c * C:(c + 1) * C])

        # ---- transposes to token-major per chunk + sumsq accumulation ----
        vtm, gtm, bctm, gam = [], [], [], []
        qraw, kraw = [], []
        ssq48 = p_small.tile([128, NCH * 6], F32, tag="ssq48")
        for c in range(NCH):
            sl = slice(c * C, (c + 1) * C)
            qt = p_raw.tile([128, NH * 128], BF16, tag="qtm")
            kt_ = p_raw.tile([128, NH * 128], BF16, tag="ktm")
            vt = p_tm.tile([128, NH * 128], BF16, tag="vtm")
            gt_ = p_tm.tile([128, NH * 128], BF16, tag="gtm")
            for j, (dst, srcs) in enumerate(((qt, XS[0:3]), (kt_, XS[3:6]),
                                             (vt, XS[6:9]), (gt_, G))):
                for jj in range(NH):
                    ps = p_pt.tile([128, 128], BF16, tag="pp", name="tps")
                    nc.tensor.transpose(ps, srcs[jj][:, sl], identb)
                    nc.vector.tensor_copy(dst[:, jj * 128:(jj + 1) * 128], ps)
            vtm.append(vt); gtm.append(gt_)
            qraw.append(qt); kraw.append(kt_)
            bt = p_tm.tile([128, 6], F32, tag="bctm")
            psb = p_pt.tile([128, 3], F32, tag="pp", name="psb")
            nc.tensor.transpose(psb, beta_fm[:, sl], identf[0:3, 0:3])
            nc.vector.tensor_copy(bt[:, 0:3], psb)
            psc = p_pt.tile([128, 3], F32, tag="pp", name="psc")
            nc.tensor.transpose(psc, cum3[:, sl], identf[0:3, 0:3])
            nc.vector.tensor_copy(bt[:, 3:6], psc)
            bctm.append(bt)
            ga = p_tm.tile([128, NH], F32, tag="gam")
            nc.scalar.activation(ga, bt[:, 3:6], AF.Exp)
            gam.append(ga)
            # sum-of-squares for l2norm: cols c*6 + (q0,q1,q2,k0,k1,k2)
            for i, (raw, jj) in enumerate([(qt, 0), (qt, 1), (qt, 2),
                                           (kt_, 0), (kt_, 1), (kt_, 2)]):
                scr = p_sc.tile([128, 128], BF16, tag="sq_scr")
                nc.scalar.activation(scr, raw[:, jj * 128:(jj + 1) * 128], AF.Square,
                                     accum_out=ssq48[:, c * 6 + i:c * 6 + i + 1])

        # batched l2norm factors: rfac = (ssq + eps) ** -0.5, all chunks at once
        lfac = p_small.tile([128, NCH * 6], F32, tag="lfac")
        nc.scalar.activation(lfac, ssq48, AF.Ln, bias=epsl2)
        rfac = p_small.tile([128, NCH * 6], F32, tag="rfac")
        nc.scalar.activation(rfac, lfac, AF.Exp, scale=-0.5)

        qn, kn = qraw, kraw
        for c in range(NCH):
            for j in range(NH):
                sl2 = slice(j * 128, (j + 1) * 128)
                nc.vector.tensor_scalar_mul(qraw[c][:, sl2], qraw[c][:, sl2],
                                            rfac[:, c * 6 + j:c * 6 + j + 1])
                nc.vector.tensor_scalar_mul(kraw[c][:, sl2], kraw[c][:, sl2],
                                            rfac[:, c * 6 + 3 + j:c * 6 + 4 + j])
            if dbg and c == 0:
                nc.sync.dma_start(dd["d_qn"], qraw[0])
                nc.sync.dma_start(dd["d_kn"], kraw[0])
                nc.sync.dma_start(dd["d_vtm"], vtm[0])

        if dbg:
            for i in range(9):
                nc.sync.dma_start(dd["d_xs"][i * 128:(i + 1) * 128, :], XS[i])
            for i in range(3):
                nc.sync.dma_start(dd["d_g"][i * 128:(i + 1) * 128, :], G[i])
            nc.sync.dma_start(dd["d_beta"], beta_fm)
            nc.sync.dma_start(dd["d_cum"], cum3)

        # ---- per-(chunk, head) precompute + chains ----
        S = []
        for j in range(NH):
            s_t = p_chain.tile([128, DVS], BF16, tag=f"S{j}")
            nc.gpsimd.memset(s_t, 0.0)
            S.append(s_t)
        ssq_sb = []

        ssq_in = p_dram.tile([T, NH], F32, tag="ssq_in")
        ssq_out = p_dram.tile([T, NH], F32, tag="ssq_out")

        for c in range(NCH):
            ssq_c = p_tm.tile([128, NH], F32, tag="ssq")
            ssq_sb.append(ssq_c)
            # E broadcast for all 3 heads of this chunk in one K=1 matmul
            e_ps = p_ps.tile([128, NH * 128], F32, tag="pp", name="e_ps")
            nc.tensor.matmul(e_ps, onesf,
                             cumcat[0:1, c * NH * 128:(c + 1) * NH * 128],
                             start=True, stop=True)
            per_j = []
            for j in range(NH):
                sl2 = slice(j * 128, (j + 1) * 128)
                gamma_col = gam[c][:, j:j + 1]
                beta_col = bctm[c][:, j:j + 1]
                cum_col = bctm[c][:, 3 + j:4 + j]

                gq = p_sc.tile([128, 128], BF16, tag="gq")
                nc.vector.tensor_scalar_mul(gq, qn[c][:, sl2], gamma_col)
                # gkv = [gamma*K | V] for the fused TK|W1 matmul
                gkv = p_sc.tile([128, 256], BF16, tag="gkv")
                nc.vector.tensor_scalar_mul(gkv[:, 0:128], kn[c][:, sl2], gamma_col)
                nc.gpsimd.tensor_copy(gkv[:, 128:256], vtm[c][:, sl2])
                kb = p_sc.tile([128, 128], BF16, tag="kb")
                nc.vector.tensor_scalar_mul(kb, kn[c][:, sl2], beta_col)

                def tr(src, tag, eng=None, ident=identb):
                    ps = p_pt.tile([128, 128], BF16, tag="pp", name="trps")
                    nc.tensor.transpose(ps, src, ident)
                    out = p_sc.tile([128, 128], BF16, tag=tag)
                    if eng == "act":
                        nc.scalar.copy(out, ps)
                    else:
                        nc.vector.tensor_copy(out, ps)
                    return out

                q_fm = tr(qn[c][:, sl2], "q_fm", eng="act")
                k_fm = tr(kn[c][:, sl2], "k_fm")
                kb_fm = tr(kb, "kb_fm")
                gq_fm = tr(gq, "gq_fm", eng="act")

                kk_ps = p_pt.tile([128, 128], F32, tag="pp")
                nc.tensor.matmul(kk_ps, k_fm, kb_fm, start=True, stop=True)
                kq_ps = p_pt.tile([128, 128], F32, tag="pp")
                nc.tensor.matmul(kq_ps, kb_fm, q_fm, start=True, stop=True)

                # x_am = (cum_j - cum_i) - mask_a ; ea = exp(-x_am)
                x_am = p_sc.tile([128, 128], F32, tag="x_am")
                nc.vector.scalar_tensor_tensor(x_am, e_ps[:, sl2], cum_col, mask_a,
                                               op0=OP.subtract, op1=OP.subtract)
                ea = p_sc.tile([128, 128], F32, tag="ea")
                nc.scalar.activation(ea, x_am, AF.Exp, scale=-1.0)
                xm = p_sc.tile([128, 128], F32, tag="xm")
                nc.vector.scalar_tensor_tensor(xm, e_ps[:, sl2], cum_col, mask_m,
                                               op0=OP.subtract, op1=OP.add)
                em = p_sc.tile([128, 128], F32, tag="em")
                nc.scalar.activation(em, xm, AF.Exp)

                B_ = p_sc.tile([128, 128], BF16, tag="B")
                nc.vector.scalar_tensor_tensor(B_, ea, -1.0, kk_ps,
                                               op0=OP.mult, op1=OP.mult)
                # mtku = [Mt | Ku] for the fused (TK^T Mt | TK^T Ku) matmul
                mtku = p_per.tile([128, 256], BF16, tag="mtku")
                nc.vector.tensor_mul(mtku[:, 0:128], em, kq_ps)
                s_col = p_sc.tile([128, 1], F32, tag="s_col")
                nc.gpsimd.tensor_mul(s_col, beta_col, em[:, 127:128])
                nc.vector.tensor_scalar_mul(mtku[:, 128:256], kn[c][:, sl2], s_col)
                gc_col = p_sc.tile([128, 1], F32, tag="gc_col")
                nc.gpsimd.tensor_mul(gc_col, em[:, 127:128], gamma_col)

                # Neumann squaring, 7 factors (I+B^(2^L)), L=0..6.
                # pair tile pr = [Bp^T | Rt]; Rt starts at I.
                # m2 = matmul(lhsT=Bp, rhs=pr) = [(Bp^2)^T | Bp^T Rt] fuses the
                # transpose-squaring with the Rt update.
                bt_ps = p_pt.tile([128, 128], BF16, tag="pp", name="btps")
                nc.tensor.transpose(bt_ps, B_, identb)
                pr = p_sc.tile([128, 256], BF16, tag="pr0")
                nc.vector.tensor_copy(pr[:, 0:128], bt_ps)
                nc.gpsimd.tensor_copy(pr[:, 128:256], identb)
                bp = B_
                for lvl in range(7):
                    bpt, rt = pr[:, 0:128], pr[:, 128:256]
                    if lvl < 6:
                        b2 = p_pt.tile([128, 128], F32, tag="pp")
                        nc.tensor.matmul(b2, bpt, bp, start=True, stop=True)
                        prp = p_pf.tile([128, 256], F32, tag="pf", name="prp")
                        nc.tensor.matmul(prp, bp, pr, start=True, stop=True)
                        pr2 = p_sc.tile([128, 256], BF16, tag=f"pr{lvl + 1}")
                        nc.vector.tensor_copy(pr2[:, 0:128], prp[:, 0:128])
                        nc.vector.tensor_add(pr2[:, 128:256], rt, prp[:, 128:256])
                        bp2 = p_sc.tile([128, 128], BF16, tag=f"bp{lvl}")
                        nc.scalar.copy(bp2, b2)
                        pr, bp = pr2, bp2
                    else:
                        pps = p_pt.tile([128, 128], F32, tag="pp")
                        nc.tensor.matmul(pps, bp, rt, start=True, stop=True)
                        rt2 = p_sc.tile([128, 128], BF16, tag="rt7")
                        nc.vector.tensor_add(rt2, rt, pps)
                tinvT = rt2

                # fused TK|W1 = Tinv @ [gamma K | V]
                tkw_ps = p_pf.tile([128, 256], F32, tag="pf", name="tkwp")
                nc.tensor.matmul(tkw_ps, tinvT, gkv, start=True, stop=True)
                tkw = p_per.tile([128, 256], BF16, tag="tkw")
                nc.vector.tensor_copy(tkw, tkw_ps)
                tk, w1 = tkw[:, 0:128], tkw[:, 128:256]
                # fused TK^T @ [Mt | Ku]
                mg_ps = p_pf.tile([128, 256], F32, tag="pf", name="mgp")
                nc.tensor.matmul(mg_ps, tk, mtku, start=True, stop=True)
                qeffT = p_per.tile([128, 128], BF16, tag="qeffT")
                nc.vector.tensor_sub(qeffT, gq_fm, mg_ps[:, 0:128])
                gt2 = p_per.tile([128, 128], BF16, tag="gt2")
                nc.vector.scalar_tensor_tensor(gt2, identb, gc_col, mg_ps[:, 128:256],
                                               op0=OP.mult, op1=OP.subtract)
                per_j.append((qeffT, gt2, mtku, w1))
                if dbg and c == 0 and j == 0:
                    nc.sync.dma_start(dd["d_e"][:, 0:128], ea)
                    nc.sync.dma_start(dd["d_e"][:, 128:256], em)
                    for di, dt_ in enumerate((tinvT, qeffT, gt2,
                                              mtku[:, 128:256], mtku[:, 0:128])):
                        nc.sync.dma_start(dd["d_per"][di * 128:(di + 1) * 128, :], dt_)
                    nc.sync.dma_start(dd["d_w1"], w1)

            # chains for chunk c
            for j in range(NH):
                qeffT, gt2, mtku, w1 = per_j[j]
                o_ps = p_pt.tile([128, DVS], F32, tag="pp")
                nc.tensor.matmul(o_ps, qeffT, S[j], start=True, stop=False)
                nc.tensor.matmul(o_ps, mtku[:, 0:128], w1, start=False, stop=True)
                s_ps = p_pt.tile([128, DVS], F32, tag="pp")
                nc.tensor.matmul(s_ps, gt2, S[j], start=True, stop=False)
                nc.tensor.matmul(s_ps, mtku[:, 128:256], w1, start=False, stop=True)
                s_new = p_chain.tile([128, DVS], BF16, tag=f"S{j}")
                nc.vector.tensor_copy(s_new, s_ps)
                S[j] = s_new
                o_sb = p_o.tile([128, DVS], BF16, tag="o_sb")
                nc.scalar.copy(o_sb, o_ps)
                if dbg:
                    nc.sync.dma_start(
                        dd["d_o"][(c * NH + j) * 128:(c * NH + j + 1) * 128, :], o_sb)
                per_j[j] = o_sb
                scr = p_sc.tile([128, DVS], BF16, tag="sq_scr2")
                nc.scalar.activation(scr, o_ps, AF.Square,
                                     accum_out=ssq_c[:, j:j + 1])
            ssq_sb[c] = (ssq_c, per_j)
            nc.sync.dma_start(ssq_in[c * 128:(c + 1) * 128, :], ssq_c)
            # staged pair AllReduce: chunks 0-5 reduce early so norm/out-proj
            # overlaps the remaining chains; chunks 6-7 reduce at the end
            if c == 5:
                nc.gpsimd.collective_compute(
                    "AllReduce", OP.add,
                    replica_groups=[[0, 1], [2, 3], [4, 5], [6, 7]],
                    ins=[ssq_in[0:768, :].opt()],
                    outs=[ssq_out[0:768, :].opt()],
                )
            elif c == 7:
                nc.gpsimd.collective_compute(
                    "AllReduce", OP.add,
                    replica_groups=[[0, 1], [2, 3], [4, 5], [6, 7]],
                    ins=[ssq_in[768:1024, :].opt()],
                    outs=[ssq_out[768:1024, :].opt()],
                )

        if dbg:
            ssqs = p_small.tile([128, NCH, NH], F32, tag="ssqs")
            nc.sync.dma_start(ssqs, ssq_out.rearrange("(c p) h -> p c h", p=128))
            nc.sync.dma_start(dd["d_ssqs"], ssqs.rearrange("p c h -> p (c h)"))

        # ---- norm * gate, transpose, out proj, reduce-scatter (per chunk) ----
        op_in = p_dram.tile([T, HID], BF16, tag="op_in")
        rs_out = p_dram.tile([T // NCORES, HID], BF16, tag="rs_out")
        for c in range(NCH):
            _, o_list = ssq_sb[c]
            sqc = p_sc.tile([128, NH], F32, tag="sqc")
            nc.sync.dma_start(sqc, ssq_out[c * 128:(c + 1) * 128, :])
            nln = p_sc.tile([128, NH], F32, tag="nln")
            nc.scalar.activation(nln, sqc, AF.Ln, scale=1.0 / DV, bias=epsn)
            nrf = p_sc.tile([128, NH], F32, tag="nrf")
            nc.scalar.activation(nrf, nln, AF.Exp, scale=-0.5)
            ogT = []
            for j in range(NH):
                o_n = p_sc.tile([128, DVS], BF16, tag="o_n")
                nc.vector.tensor_scalar_mul(o_n, o_list[j],
                                            nrf[:, j:j + 1])
                o_g = p_sc.tile([128, DVS], BF16, tag="o_g")
                nc.vector.tensor_mul(o_g, o_n, gtm[c][:, j * 128:(j + 1) * 128])
                ps = p_pt.tile([128, 128], BF16, tag="pp", name="ogps")
                nc.tensor.transpose(ps, o_g, identb)
                og = p_sc.tile([128, 128], BF16, tag=f"ogT{j}")
                nc.vector.tensor_copy(og, ps)
                ogT.append(og)
            outp = p_out.tile([128, HID], BF16, tag="outp")
            for nt in range(4):
                ps = p_ps.tile([128, 512], F32, tag="pp")
                for j in range(NH):
                    nc.tensor.matmul(ps, ogT[j], wo_sb[j][:, nt * 512:(nt + 1) * 512],
                                     start=(j == 0), stop=(j == NH - 1))
                nc.scalar.copy(outp[:, nt * 512:(nt + 1) * 512], ps)
            nc.sync.dma_start(op_in[c * 128:(c + 1) * 128, :], outp)
            if dbg:
                nc.sync.dma_start(dd["d_op"][c * 128:(c + 1) * 128, :], outp)

        nc.gpsimd.collective_compute(
            "ReduceScatter", OP.add,
            replica_groups=[list(range(NCORES))],
            ins=[op_in.opt()], outs=[rs_out.opt()],
        )
        nc.sync.dma_start(d_out, rs_out[:])

    nc.compile()
    return nc


_BUILT = None


def _get_built():
    global _BUILT
    if _BUILT is None:
        _BUILT = build()
    return _BUILT


def _prep_in_maps(hidden_states, Wq, Wk, Wv, Wb, Wa, Wg, Wo,
                  conv_wq, conv_wk, conv_wv, A_log, dt_bias, norm_w):
    hsT = np.ascontiguousarray(hidden_states[0].astype(np.float32).T).astype(BF)
    Wq_b, Wk_b = Wq.astype(BF), Wk.astype(BF)
    Wv_b, Wg_b = Wv.astype(BF), Wg.astype(BF)
    Wb_b, Wa_b = Wb.astype(BF), Wa.astype(BF)
    Wo_eff = (np.tile(norm_w.astype(np.float32), H)[:, None] * Wo).astype(BF)
    cq = conv_wq.astype(np.float32)
    ck = conv_wk.astype(np.float32)
    cv = conv_wv.astype(np.float32)

    ii, jj = np.indices((128, 128))
    mask_a = np.where(jj < ii, 0.0, NEG).astype(np.float32)   # strict lower keep
    mask_m = np.where(jj >= ii, 0.0, NEG).astype(np.float32)  # upper incl diag keep
    masks = np.ascontiguousarray(np.concatenate([mask_a, mask_m], axis=1))
    identb = np.eye(128, dtype=np.float32).astype(BF)
    identf = np.eye(128, dtype=np.float32)
    onesf = np.ones((1, 128), np.float32)

    in_maps = []
    for core in range(NCORES):
        g_, d = divmod(core, 2)
        heads = [3 * g_, 3 * g_ + 1, 3 * g_ + 2]
        qk_idx = np.concatenate([np.arange(h * DK, (h + 1) * DK) for h in heads])
        v_idx = np.concatenate(
            [np.arange(h * DV + d * DVS, h * DV + d * DVS + DVS) for h in heads])
        w = np.concatenate([Wq_b[:, qk_idx], Wk_b[:, qk_idx],
                            Wv_b[:, v_idx], Wg_b[:, v_idx],
                            Wb_b[:, heads], Wa_b[:, heads]], axis=1)
        convw = np.ascontiguousarray(
            np.concatenate([cq[qk_idx], ck[qk_idx], cv[v_idx]], axis=0))
        scal = np.stack([-np.exp(A_log[heads].astype(np.float32)),
                         dt_bias[heads].astype(np.float32)], axis=1)
        in_maps.append({
            "hsT": hsT, "w": np.ascontiguousarray(w),
            "wo": np.ascontiguousarray(Wo_eff[v_idx]),
            "convw": convw, "masks": masks, "identb": identb,
            "identf": identf, "onesf": onesf,
            "scal": np.ascontiguousarray(scal),
        })
    return in_maps


def _run(in_maps, trace=False):
    nc = _get_built()
    return run_bass_kernel_spmd(nc, in_maps, list(range(NCORES)), trace=trace)


def kernel(**inputs):
    in_maps = _prep_in_maps(**inputs)
    res = _run(in_maps, trace=False)
    out = np.concatenate([np.asarray(res.results[i]["out"]) for i in range(NCORES)],
                         axis=0).astype(np.float32)
    return out.reshape(1, T, HID)


_BUILT_DBG = None


def kernel_debug(**inputs):
    global _BUILT_DBG
    if _BUILT_DBG is None:
        _BUILT_DBG = build(dbg=True)
    in_maps = _prep_in_maps(**inputs)
    res = run_bass_kernel_spmd(_BUILT_DBG, in_maps, list(range(NCORES)))
    return res.results


def _ensure_ntff_hook():
    import sys as _sys
    import types as _types
    try:
        from antenv.axon_hooks import get_axon_ntff_profile_hook  # noqa: F401
        return
    except ImportError:
        pass
    from trn_agent_boot.trn_boot import _ntff_profile_via_ctypes
    hook = _ntff_profile_via_ctypes('/opt/axon/libaxon_pjrt.so')
    mod = _types.ModuleType("antenv.axon_hooks")
    mod.get_axon_ntff_profile_hook = lambda: hook
    mod.set_axon_ntff_profile_hook = lambda h: None
    _sys.modules["antenv.axon_hooks"] = mod
    import antenv
    antenv.axon_hooks = mod
    # avoid artifact upload (no bucket access in this env)
    import concourse.bass_utils as _bu
    _bu.upload_artifacts = lambda d: ""


def kernel_traced(**inputs):
    _ensure_ntff_hook()
    in_maps = _prep_in_maps(**inputs)
    res = _run(in_maps, trace=True)
    out = np.concatenate([np.asarray(res.results[i]["out"]) for i in range(NCORES)],
                         axis=0).astype(np.float32)
    return out.reshape(1, T, HID), res
